# revision 13
# baseline (speedup 1.0000x reference)
"""Multi-head attention forward, sharded head-parallel across 8 NeuronCores.

Per core c (heads 2c, 2c+1), fp16 data path (fp8 adds ~3% error -- input
quantization noise scales WITH the signal through the contractions, it
does not average away):
  qT/kT/vT = (x @ W{q,k,v}_c.T).T       W.T-tiled matmuls vs fp16 xT,
                                        sequential q/k/v PSUM accumulation
  v1       = paired [128,128] PE transposes of vT (both heads at once),
             layout [kpos, KC, 2, (v_h|1)] with interleaved ones columns
  scoresT  = kT_chunk.T @ qT            [k-pos partitions, q-pos free],
             two heads on PE row-groups 0-63/64-127, diag blocks
             column-trimmed, ascending j so j=0 is full width
  probsT   = exp(scoresT) fp16, diag triangle zeroed by tri-mask mul
  av+rowsum: pos = [v_h | 1].T @ probsT (ones column yields softmax denom)
  normalize: batched denom copy -> one reciprocal (DVE, staged through
             SBUF -- custom DVE ops read garbage from PSUM on HW),
             partition_broadcast (gpsimd), multiply pos x bs -> ocat
  out_projT partial per 512-tile, fp16 partials DMA'd out
Host: sum the 8 partial [1024, 4096] fp16 outputs in fp32, transpose, bias.

Perf structure (the previous flat emission ran 168 us; PE busy was only
76% with p-state degradation from stalls -- TRN2's PE needs >3us of
continuous execution to reach 2.4 GHz):
  - stage_a tiles 1-7, transposes, and out-proj tiles are emitted as
    FILLER inside the attention windows so the PE priority queue always
    has independent work during the scores->exp->av dependency chain.
  - DMA descriptor issuance is spread across engines (gpsimd DIRECT2D
    serialization previously delayed the first data DMA to 10.4us).
  - psA/psO unified into one 4-slot PSUM bank ring so the per-t
    normalize chain does not gate the next window's AV accumulation.
  - 8 PSUM banks: ring 4 x [128,512]f32 + scores 2 x [128,2,512]f32.
"""
import sys

sys.path.insert(0, "/opt/trn_rl_repo")

from collections import deque

import ml_dtypes
import numpy as np

F16 = np.float16
F8 = ml_dtypes.float8_e4m3

B, S, D = 2, 2048, 1024
H, HD = 16, 64
NCORES = 8
SEC = 128           # output dims per core per section (2 heads * 64)
BS = B * S          # 4096
NT = BS // 512      # 8 seq tiles of 512
EC = D // 128       # 8 embed chunks
QT = S // 512       # 4 q-tiles per (b,h)
KC = S // 128       # 16 k-chunks per (b,h)

_cache = {}


def _build(mask_mode):
    import concourse.bass as bass
    import concourse.tile as tile
    from concourse import bacc, mybir

    f32 = mybir.dt.float32
    f16 = mybir.dt.float16
    Exp = mybir.ActivationFunctionType.Exp

    nc = bacc.Bacc("TRN2", target_bir_lowering=False, debug=False,
                   num_devices=NCORES)

    xT = nc.dram_tensor("xT", [D, BS], f16, kind="ExternalInput")
    wqkvT = nc.dram_tensor("wqkvT", [D, 3 * SEC], f16, kind="ExternalInput")
    woT = nc.dram_tensor("woT", [SEC, D], f16, kind="ExternalInput")
    # consts: [:, 0:128] = eye(128); [:, 128:384] = tri duplicated (2x128)
    consts = nc.dram_tensor("consts", [128, 384], f16, kind="ExternalInput")
    if mask_mode == "general":
        maskT = nc.dram_tensor("maskT", [S, S], f16, kind="ExternalInput")
    out_pT = nc.dram_tensor("out_pT", [D, BS], f16, kind="ExternalOutput")

    causal = mask_mode == "causal"

    with tile.TileContext(nc) as tc:
        with (
            nc.allow_low_precision(reason="fp16 attention pipeline"),
            tc.tile_pool(name="singles", bufs=1) as singles,
            tc.tile_pool(name="qkv", bufs=1) as qkv,
            tc.tile_pool(name="prp", bufs=4) as prp,
            tc.tile_pool(name="mskp", bufs=4) as mskp,
            tc.tile_pool(name="nrm", bufs=4) as nrm,
            tc.tile_pool(name="ftp", bufs=2) as ftp,
            tc.tile_pool(name="psA", bufs=2, space="PSUM") as psA,
            tc.tile_pool(name="psO", bufs=2, space="PSUM") as psO,
            tc.tile_pool(name="psS", bufs=2, space="PSUM") as psS,
        ):
            # ---- static tiles ----
            w_sb = singles.tile([128, EC, 3 * SEC], f16)
            woT_sb = singles.tile([128, D], f16)
            ident = singles.tile([128, 128], f16)
            if causal:
                tri2 = singles.tile([128, 2, 128], f16)
            xfull = qkv.tile([128, EC, BS], f16)

            # ---- DMA prefetch: spread issuance across engines so the
            # first stage_a matmul isn't gated on serialized descriptor
            # writes; most-urgent data first ----
            wqr = wqkvT.rearrange("(ec p) c -> p ec c", p=128)
            xTr = xT.rearrange("(ec p) s -> p ec s", p=128)
            eng3 = (nc.sync, nc.scalar, nc.gpsimd)
            # x tile 0 in two halves (ec 0-3, 4-7) on two queues
            nc.sync.dma_start(out=xfull[:, 0:4, 0:512],
                              in_=xTr[:, 0:4, 0:512])
            nc.scalar.dma_start(out=xfull[:, 4:8, 0:512],
                               in_=xTr[:, 4:8, 0:512])
            for ec in range(EC):
                eng3[ec % 2].dma_start(out=w_sb[:, ec, :], in_=wqr[:, ec, :])
            nc.gpsimd.dma_start(out=ident[:], in_=consts[:, 0:128])
            if causal:
                nc.gpsimd.dma_start(
                    out=tri2[:],
                    in_=consts[:, 128:384].rearrange("p (h c) -> p h c", h=2))
            for n in range(1, NT):
                sl = slice(512 * n, 512 * (n + 1))
                eng3[n % 3].dma_start(out=xfull[:, :, sl], in_=xTr[:, :, sl])
            nc.gpsimd.dma_start(out=woT_sb[:], in_=woT[:])

            qT = qkv.tile([128, BS], f16)
            kT = qkv.tile([128, BS], f16)
            vT = qkv.tile([128, BS], f16)
            ocat = qkv.tile([128, BS], f16)
            v1s = []
            for b in range(B):
                v1 = qkv.tile([128, KC, 2, 65], f16, name=f"v1_{b}")
                v1s.append(v1)

            dsts = (qT, kT, vT)

            # ---- unit factories (each emits a small independent chunk
            # of work; used directly or queued as window filler) ----

            def a_unit(n, part):
                """One qkv-projection part: 8 acc matmuls + 1 copy."""
                nsl = slice(512 * n, 512 * (n + 1))
                csl = slice(128 * part, 128 * (part + 1))
                pa = psA.tile([128, 512], f32, tag="pa", name="pa")
                for ec in range(EC):
                    nc.tensor.matmul(
                        pa[:], w_sb[:, ec, csl], xfull[:, ec, nsl],
                        start=ec == 0, stop=ec == EC - 1)
                nc.any.tensor_copy(dsts[part][:, nsl], pa[:])

            def t_unit(b, chunks):
                """PE transposes of vT chunks into v1 (both heads)."""
                base = S * b
                v1 = v1s[b]
                for i in chunks:
                    pt = psA.tile([128, 1024], f16, tag="pa", name="pt")
                    nc.tensor.transpose(
                        pt[:, 0:128],
                        vT[:, base + 128 * i:base + 128 * (i + 1)], ident[:])
                    nc.any.tensor_copy(
                        v1[:, i, :, 0:64],
                        pt[:, 0:128].rearrange("p (h c) -> p h c", h=2))

            out_r = out_pT.rearrange("(oc p) s -> p oc s", p=128)
            _c_dma_eng = [nc.sync, nc.scalar, nc.gpsimd, nc.sync]

            def c_units(n):
                """Out-proj for one 512-pos tile as 4 filler units."""
                ssl = slice(512 * n, 512 * (n + 1))
                ft_box = []

                def mk(ocs, last):
                    def run():
                        if not ft_box:
                            ft_box.append(ftp.tile(
                                [128, EC, 512], f16, tag="ft", name="ft"))
                        ft = ft_box[0]
                        for oc in ocs:
                            osl = slice(128 * oc, 128 * (oc + 1))
                            pf = psA.tile([128, 512], f32, tag="pa",
                                          name="pf")
                            nc.tensor.matmul(pf[:], woT_sb[:, osl],
                                             ocat[:, ssl], start=True,
                                             stop=True)
                            nc.any.tensor_copy(ft[:, oc, :], pf[:])
                        if last:
                            # split the store across two queues
                            e0 = _c_dma_eng[n % 4]
                            e1 = _c_dma_eng[(n + 2) % 4]
                            e0.dma_start(out=out_r[:, 0:4, ssl],
                                         in_=ft[:, 0:4, :])
                            e1.dma_start(out=out_r[:, 4:8, ssl],
                                         in_=ft[:, 4:8, :])
                    return run

                return [mk((0, 1), False), mk((2, 3), False),
                        mk((4, 5), False), mk((6, 7), True)]

            def b_window(b, t, fill):
                """Attention for q-tile t of batch b; drains `fill`
                (deque of closures) evenly across the j loop."""
                base = S * b
                v1 = v1s[b]
                qsl = slice(base + 512 * t, base + 512 * (t + 1))
                njc = 4 * t + 4 if causal else KC
                nfill = len(fill)
                drained = 0
                pos = [psO.tile([65, 512], f32, tag="po",
                                name=f"po{b}{t}{lh}") for lh in range(2)]
                for j in range(njc):
                    jm = j - 4 * t
                    c0 = 128 * jm if causal and jm >= 0 else 0
                    ksl = slice(base + 128 * j, base + 128 * (j + 1))
                    ps = psS.tile([128, 2, 512], f32, tag="ps", name="ps")
                    for lh in range(2):
                        hsl = slice(64 * lh, 64 * (lh + 1))
                        nc.tensor.matmul(
                            ps[:, lh, c0:], kT[hsl, ksl],
                            qT[hsl, qsl.start + c0:qsl.stop],
                            start=True, stop=True)
                    pr = prp.tile([128, 2, 512], f16, tag="pr", name="pr")
                    nc.scalar.activation(pr[:, :, c0:], ps[:, :, c0:], Exp)
                    if causal and jm >= 0:
                        win = slice(c0, c0 + 128)
                        nc.any.tensor_mul(pr[:, :, win], pr[:, :, win],
                                          tri2[:])
                    elif mask_mode == "general":
                        msk = mskp.tile([128, 512], f16, tag="mk",
                                        name="msk")
                        nc.sync.dma_start(
                            out=msk[:],
                            in_=maskT[128 * j:128 * (j + 1),
                                      512 * t:512 * (t + 1)])
                        for lh in range(2):
                            nc.any.tensor_mul(pr[:, lh, :], pr[:, lh, :],
                                              msk[:])
                    for lh in range(2):
                        nc.tensor.matmul(
                            pos[lh][:, c0:], v1[:, j, lh, :],
                            pr[:, lh, c0:],
                            start=j == 0, stop=j == njc - 1)
                    # drain filler evenly across the window
                    want = nfill * (j + 1) // njc
                    while drained < want:
                        fill.popleft()()
                        drained += 1
                while fill:
                    fill.popleft()()
                # ---- normalize: stage pos through SBUF (frees the psO
                # bank early AND custom DVE ops read garbage from PSUM),
                # batched denoms -> one reciprocal ----
                psb = [nrm.tile([65, 512], f32, tag=f"psb{lh}",
                                name=f"psb{lh}") for lh in range(2)]
                for lh in range(2):
                    nc.any.tensor_copy(psb[lh][:], pos[lh][:])
                for lh in range(2):
                    hsl = slice(64 * lh, 64 * (lh + 1))
                    dn = nrm.tile([1, 512], f32, tag=f"dn{lh}",
                                  name=f"dn{lh}")
                    nc.vector.tensor_copy(dn[:], psb[lh][64:65, :])
                    rc = nrm.tile([1, 512], f32, tag=f"rc{lh}",
                                  name=f"rc{lh}")
                    nc.vector.reciprocal_approx_fast(rc[:], dn[:])
                    bs_ = nrm.tile([64, 512], f32, tag="bs", name="bs")
                    nc.gpsimd.partition_broadcast(bs_[:], rc[:])
                    nc.any.tensor_mul(ocat[hsl, qsl], psb[lh][0:64, :],
                                      bs_[:])

            # ---- schedule ----
            # window (b,t) -> filler emitted inside it
            for part in range(3):
                a_unit(0, part)
            nc.vector.memset(v1s[0][:, :, :, 64], 1.0)
            t_unit(0, range(0, 4))

            def fa(n):
                return [lambda p=p: a_unit(n, p) for p in range(3)]

            def ft_(b, chunks, pre=None):
                def run():
                    if pre is not None:
                        pre()
                    t_unit(b, chunks)
                return [run]

            b_window(0, 0, deque(fa(1) + ft_(0, range(4, 8))))
            b_window(0, 1, deque(fa(2) + ft_(0, range(8, 12))))
            b_window(0, 2, deque(fa(3) + ft_(0, range(12, 16))))
            b_window(0, 3, deque(
                fa(4) + ft_(1, range(0, 4),
                            pre=lambda: nc.vector.memset(
                                v1s[1][:, :, :, 64], 1.0))))
            b_window(1, 0, deque(fa(5) + ft_(1, range(4, 8)) + c_units(0)))
            b_window(1, 1, deque(fa(6) + ft_(1, range(8, 12)) + c_units(1)))
            b_window(1, 2, deque(fa(7) + ft_(1, range(12, 16))
                                 + c_units(2) + c_units(4)))
            b_window(1, 3, deque(c_units(3) + c_units(5) + c_units(6)))
            for u in c_units(7):
                u()

    nc.compile()
    return nc


def _classify_mask(mask):
    m = np.asarray(mask).reshape(S, S) != 0
    if m.all():
        return "none", None
    if np.array_equal(m, np.tril(np.ones((S, S), bool))):
        return "causal", None
    return "general", m.T.astype(np.float32)


def _ensure_ntff_hook():
    """Register antenv.axon_hooks with a ctypes NTFF profile hook if the
    container image lacks it (mirrors trn_agent_boot's registration)."""
    import types
    try:
        from antenv.axon_hooks import get_axon_ntff_profile_hook  # noqa: F401
        return
    except ImportError:
        pass
    import contextlib
    import ctypes

    hook = None
    so_path = "/opt/axon/libaxon_pjrt.so"
    try:
        lib = ctypes.CDLL(so_path)
        if hasattr(lib, "axon_start_nrt_profile"):
            lib.axon_start_nrt_profile.argtypes = [
                ctypes.POINTER(ctypes.c_int64), ctypes.c_size_t]
            lib.axon_start_nrt_profile.restype = ctypes.c_int64
            lib.axon_stop_nrt_profile.argtypes = [ctypes.c_char_p]
            lib.axon_stop_nrt_profile.restype = ctypes.c_int64

            @contextlib.contextmanager
            def _hook(output_dir, device_ids):
                import jax
                jax.devices()
                if device_ids:
                    ids = (ctypes.c_int64 * len(device_ids))(*device_ids)
                    rc = lib.axon_start_nrt_profile(ids, len(device_ids))
                else:
                    rc = lib.axon_start_nrt_profile(None, 0)
                if rc != 0:
                    raise RuntimeError(f"axon_start_nrt_profile rc={rc}")
                try:
                    yield
                finally:
                    n = lib.axon_stop_nrt_profile(str(output_dir).encode())
                    print(f"profile: {n} file(s) written to {output_dir}",
                          flush=True)

            hook = _hook
    except OSError:
        pass

    mod = types.ModuleType("antenv.axon_hooks")
    _h = [hook]
    mod.get_axon_ntff_profile_hook = lambda: _h[0]

    def _set(h):
        _h[0] = h

    mod.set_axon_ntff_profile_hook = _set
    sys.modules["antenv.axon_hooks"] = mod
    try:
        import antenv
        antenv.axon_hooks = mod
    except ImportError:
        pass


def kernel(key, query, value, mask, W_qkv, W_out, b_out):
    from concourse.bass_utils import run_bass_kernel_spmd
    import os

    mask_mode, maskT = _classify_mask(mask)
    if mask_mode not in _cache:
        _cache[mask_mode] = _build(mask_mode)
    nc = _cache[mask_mode]

    x = np.ascontiguousarray(
        np.asarray(query, np.float32).reshape(BS, D))
    xT_f16 = np.ascontiguousarray(x.T).astype(F16)
    W_qkv = np.asarray(W_qkv, np.float32)
    W_out = np.asarray(W_out, np.float32)

    consts = np.zeros((128, 384), F16)
    consts[:, 0:128] = np.eye(128, dtype=F16)
    tri = (np.arange(128)[:, None] <= np.arange(128)[None, :]).astype(F16)
    consts[:, 128:256] = tri
    consts[:, 256:384] = tri

    in_maps = []
    for c in range(NCORES):
        sl = slice(SEC * c, SEC * (c + 1))
        wq = W_qkv[sl, :].T * np.float32(HD ** -0.5)
        wk = W_qkv[D + SEC * c:D + SEC * (c + 1), :].T
        wv = W_qkv[2 * D + SEC * c:2 * D + SEC * (c + 1), :].T
        m = {
            "xT": xT_f16,
            "consts": consts,
            "wqkvT": np.ascontiguousarray(np.concatenate(
                [wq, wk, wv], axis=1, dtype=np.float32)).astype(F16),
            "woT": np.ascontiguousarray(W_out[:, sl].T).astype(F16),
        }
        if mask_mode == "general":
            m["maskT"] = maskT.astype(F16)
        in_maps.append(m)

    trace = bool(int(os.environ.get("KERNEL_TRACE", "0")))
    if trace:
        _ensure_ntff_hook()
        try:
            res = run_bass_kernel_spmd(nc, in_maps,
                                       core_ids=list(range(NCORES)),
                                       trace=True)
        except Exception as e:
            print(f"traced run failed ({e!r}); retrying untraced",
                  flush=True)
            res = run_bass_kernel_spmd(nc, in_maps,
                                       core_ids=list(range(NCORES)))
        print(f"HW exec time: {res.exec_time_ns} ns", flush=True)
        kernel.last_exec_ns = res.exec_time_ns
        kernel.last_results = res
    else:
        res = run_bass_kernel_spmd(nc, in_maps, core_ids=list(range(NCORES)))
        kernel.last_results = res

    acc = res.results[0]["out_pT"].astype(np.float32)
    for c in range(1, NCORES):
        acc = acc + res.results[c]["out_pT"]
    out = acc.T.reshape(B, S, D) + np.asarray(b_out, np.float32)
    return out.astype(np.float32)
